# revision 38
# baseline (speedup 1.0000x reference)
"""Trainium2 Bass kernel for nn_AttentionBlock (causal attention block), v2.

Self-contained: takes FULL inputs (batch 32), shards batch over 8 NeuronCores
(4 samples/core, pure data parallel), runs a Bass/Tile kernel per core, and
gathers the full [32, 160, 32, 32] output.

v2 design (vs the fp32r baseline):
- bf16 matmuls and bf16 SBUF data everywhere (rel-err budget 2e-2 allows it):
  PE runs at 1 cycle/row instead of fp32r's ~3, and DVE element-wise ops get
  the 16-bit 2x mode.
- no identity-copy of nin1 outputs into fp32: h is copied PSUM->SBUF bf16 once
  (ACT identity + bias, PSUM sources may shift partitions), then all elu math
  runs 1024-wide on bf16 SBUF tiles.
- elu decomposition per sign, from m = min(h,0), rp = relu(h):
    stream_pos = exp(m) + rp        stream_neg = exp(-rp) - m
  (exp on ScalarE; min/max maps + adds on DVE/GPSIMD per ENG table).
- gate: nin2 out layout [gb | pad | 0.5*ga]; T = tanh(0.5*gb + 0.5*b_gb) + 1;
  G = (0.5*ga + 0.5*b_ga) * T.  The grn residual (+C) for the k/q/v GRNs is
  folded into the K/Q/V projection matmuls (proj(G) + proj(C)); the output
  GRN adds ul explicitly.
- K/Q/V projections run as two accumulated PSUM sets ([K|pad|Q] 48 rows, [V]
  80 rows) sharing the C-residual matmuls.
- attention identical in structure to baseline (S^T per k-tile, exp without
  max-subtraction, ones-row in V^T for free softmax denominators), in bf16.
"""

import sys

sys.path.insert(0, "/opt/trn_rl_repo")

import contextlib

import ml_dtypes
import numpy as np

import concourse.bacc as bacc
import concourse.mybir as mybir
from concourse.bass_utils import run_bass_kernel_spmd
from concourse.tile import TileContext

F32 = mybir.dt.float32
BF16 = mybir.dt.bfloat16
AF = mybir.ActivationFunctionType
OP = mybir.AluOpType
BF = ml_dtypes.bfloat16

N, XD, NF = 32, 3, 160
KD, VD = 16, 80
CK, CQ = 169, 166
HWP = 1024
NS = 4  # samples per core
NCORES = 8
EPS = 1e-7
PAD = 192  # elu- stream offset

# v3: single-exp elu streams.  From e = exp(-|h|):
#   pos = elu(h)+1  = max(h+1, e)   (exp(x) >= 1+x, equality at 0)
#   neg = elu(-h)+1 = max(1-h, e)
# One ACT pass per elu instead of two; DVE work uses tensor_scalar (4x mode)
# and tensor_tensor (2x) instead of scalar_tensor_tensor (no fast mode).


def chunks(total, step=128):
    return [(o, min(step, total - o)) for o in range(0, total, step)]


_PLIMIT = {0: 128, 32: 32, 64: 64, 96: 32}


def legal_segs(src_off, dst_off, length, src_sbuf=True):
    """Split a row-range copy into SBUF-legal pieces (windows at 0/32/64/96).
    PSUM sources are exempt.  Yields (src_tile, src_row, dst_tile, dst_row, L).
    """
    done = 0
    while done < length:
        s, d = src_off + done, dst_off + done
        sb, db = s % 128, d % 128
        L = min(_PLIMIT[db], 128 - db, length - done)
        if src_sbuf:
            L = min(L, _PLIMIT[sb], 128 - sb)
        else:
            L = min(L, 128 - sb)
        yield (s // 128, sb, d // 128, db, L)
        done += L


# ---------------------------------------------------------------- host prep --


def bias_chunked(bias):
    nm = (len(bias) + 127) // 128
    t = np.zeros((128, nm), np.float32)
    for m in range(nm):
        seg = bias[128 * m : 128 * (m + 1)]
        t[: len(seg), m] = seg
    return t


def prep_weights(inp):
    """Numpy prep: permutations, stream packing, bias folds, 0.5 gate scaling.

    Channel order 'cb' = [ul(160), b(6), x(3)].  Streams (matmul rhs rows):
    [elu+ (C) | pad->192 | elu- (C)].  nin2 out layout [gb | pad | 0.5*ga].
    Streams hold elu(x)+1, so each consumer's bias folds -W.sum(1).
    """
    p = {}
    perm_k = np.array(list(range(3, 169)) + list(range(0, 3)))
    perm_q = np.arange(166)

    def stream_cols(Wi, perm):
        C = Wi.shape[1] // 2
        W1, W2 = Wi[:, :C][:, perm], Wi[:, C:][:, perm]
        out = np.zeros((Wi.shape[0], 361), np.float32)
        out[:, : W1.shape[1]] = W1
        out[:, PAD : PAD + W2.shape[1]] = W2
        return out, Wi.sum(1)

    kW, kfold = stream_cols(inp["gkWi"], perm_k)
    vW, vfold = stream_cols(inp["gvWi"], perm_k)
    qW, qfold = stream_cols(inp["gqWi"], perm_q)
    Wab = np.zeros((550, 361), np.float32)
    Wab[0:169] = kW
    Wab[192:361] = vW
    Wab[384:550] = qW
    bab = np.zeros(550, np.float32)
    bab[0:169] = inp["gkbi"] - kfold
    bab[192:361] = inp["gvbi"] - vfold
    bab[384:550] = inp["gqbi"] - qfold
    p["wab_t"] = Wab.T  # [361, 550]
    p["bab"] = bias_chunked(bab)

    def inner_w(Wo, bo, out_perm):
        """nin2 lhsT [stream rows, out rows] with out layout [ga05|pad|gb]:
        ga at rows [0,C) so the DVE gate STT drains in 2 aligned segments;
        gb (tanh, on ACT) takes the fragmented rows [PAD, PAD+C).
        bias tile holds 0.5*b_gb at gb rows (for the tanh AP bias)."""
        C = Wo.shape[1] // 2
        W1, W2 = Wo[:, :C], Wo[:, C:]
        bias = bo - (W1.sum(1) + W2.sum(1))
        Wfull = np.concatenate([W1, W2], axis=1)  # [2C out, 2C in]
        gb_w = Wfull[C + out_perm]
        ga_w = Wfull[out_perm] * 0.5
        n = PAD + C
        Ws = np.zeros((n, n), np.float32)
        for rows, w_ in ((slice(0, C), ga_w), (slice(PAD, n), gb_w)):
            Ws[rows, 0:C] = w_[:, 0:C]
            Ws[rows, PAD : PAD + C] = w_[:, C : 2 * C]
        bs = np.zeros(n, np.float32)
        bs[PAD:n] = 0.5 * bias[C + out_perm]
        # ga-part bias indexed by destination row (channel), for the gate STT
        bga = np.zeros(256, np.float32)
        bga[0:C] = 0.5 * bias[out_perm]
        return Ws.T, bias_chunked(bs), bias_chunked(bga)

    p["wok_t"], p["bok"], p["bgk"] = inner_w(inp["gkWo"], inp["gkbo"], perm_k)
    p["woq_t"], p["boq"], p["bgq"] = inner_w(inp["gqWo"], inp["gqbo"], perm_q)
    p["wov_t"], p["bov"], p["bgv"] = inner_w(inp["gvWo"], inp["gvbo"], perm_k)
    p["woo_t"], p["boo"], p["bgo"] = inner_w(inp["goWo"], inp["gobo"], np.arange(NF))

    # K/Q/V projections with folded +C residual.
    nk = inp["nkW"][:, perm_k]  # [16, 169]
    nq = inp["nqW"][:, perm_q]  # [16, 166]
    nv = inp["nvW"][:, perm_k]  # [80, 169]
    pjk = np.zeros((CK, 48), np.float32)
    pjk[:, 0:16] = nk.T
    pjq = np.zeros((CQ, 48), np.float32)
    pjq[:, 32:48] = nq.T
    pjc = np.zeros((CK, 48), np.float32)
    pjc[:, 0:16] = nk.T
    pjc[0:CQ, 32:48] = nq.T
    p["pjk"], p["pjq"], p["pjc"] = pjk, pjq, pjc
    p["pjv"] = np.ascontiguousarray(nv.T)  # used for both G_v and C chunks
    njb = np.zeros((128, 2), np.float32)
    njb[0:16, 0] = inp["nkb"]
    njb[32:48, 0] = inp["nqb"]
    njb[0:80, 1] = inp["nvb"]
    p["njb"] = njb

    # grn_out nin1: Sc(ul rows) + Sa(att stream) -> h_o [160]
    W1, W2 = inp["goWi"][:, :NF], inp["goWi"][:, NF:]
    wa = np.zeros((NF, 361), np.float32)
    wa[:, 0:NF] = W1
    wa[:, PAD : PAD + NF] = W2
    p["wo1a_t"] = wa.T  # [361, 160]
    p["wo1bp"] = np.ascontiguousarray(inp["goWs"][:, :VD].T)  # [80, 160]
    p["wo1bn"] = np.ascontiguousarray(inp["goWs"][:, VD:].T)
    p["bo1"] = bias_chunked(
        (inp["gobi"] + inp["gobs"] - inp["goWi"].sum(1) - inp["goWs"].sum(1)).astype(
            np.float32
        )
    )

    pp = np.arange(128)[:, None]
    ff = np.arange(128)[None, :]
    p["masks"] = (ff > pp).astype(np.float32)
    p["eps_nzq"] = (EPS * (HWP - np.arange(HWP, dtype=np.float32)))[None, :]
    p["ident80"] = np.eye(80, dtype=np.float32)

    out = {}
    for k, v in p.items():
        dt = np.float32 if k in F32_W else BF
        out[k] = np.ascontiguousarray(np.asarray(v, dtype=np.float32).astype(dt))
    return out


WSHAPES = {
    "wab_t": (361, 550),
    "wok_t": (361, 361),
    "woq_t": (358, 358),
    "wov_t": (361, 361),
    "woo_t": (352, 352),
    "wo1a_t": (361, 160),
    "wo1bp": (80, 160),
    "wo1bn": (80, 160),
    "pjk": (169, 48),
    "pjq": (166, 48),
    "pjc": (169, 48),
    "pjv": (169, 80),
    "bab": (128, 5),
    "bok": (128, 3),
    "bgk": (128, 2),
    "bgq": (128, 2),
    "bgv": (128, 2),
    "bgo": (128, 2),
    "boq": (128, 3),
    "bov": (128, 3),
    "boo": (128, 3),
    "bo1": (128, 2),
    "njb": (128, 2),
    "masks": (128, 128),
    "eps_nzq": (1, HWP),
    "ident80": (80, 80),
}
F32_W = {"bab", "bok", "boq", "bov", "boo", "bo1", "njb", "eps_nzq", "bgk", "bgq", "bgv", "bgo"}


def _pack_layout():
    """Column layout packing every weight chunk side-by-side into one bf16
    and one f32 [128, *] tensor (single DMA each).  eps_nzq stays separate
    (it is [1, 1024])."""
    layout = {}
    offs = {"b": 0, "f": 0}
    for k, (rows, width) in WSHAPES.items():
        if k == "eps_nzq":
            continue
        which = "f" if k in F32_W else "b"
        entries = []
        for o, L in chunks(rows):
            entries.append((L, offs[which]))
            offs[which] += width
        layout[k] = (which, width, entries)
    return layout, offs["b"], offs["f"]


PACK_LAYOUT, PACKB_W, PACKF_W = _pack_layout()


def pack_weights(p):
    wpb = np.zeros((128, PACKB_W), BF)
    wpf = np.zeros((128, PACKF_W), np.float32)
    for k, (which, width, entries) in PACK_LAYOUT.items():
        arr = p[k]
        dst = wpb if which == "b" else wpf
        for (o, L), (L2, off) in zip(chunks(WSHAPES[k][0]), entries):
            dst[0:L, off : off + width] = arr[o : o + L, :]
    return wpb, wpf


def build_nc(ns=NS):
    nc = bacc.Bacc("TRN2", target_bir_lowering=False, debug=False)

    x_d = nc.dram_tensor("x", [ns, XD, HWP], BF16, kind="ExternalInput")
    ul_d = nc.dram_tensor("ul", [ns, NF, HWP], BF16, kind="ExternalInput")
    b_d = nc.dram_tensor("b", [ns, 2 * XD, HWP], BF16, kind="ExternalInput")
    out_d = nc.dram_tensor("out", [ns, NF, HWP], F32, kind="ExternalOutput")
    wpb_d = nc.dram_tensor("wpb", [128, PACKB_W], BF16, kind="ExternalInput")
    wpf_d = nc.dram_tensor("wpf", [128, PACKF_W], F32, kind="ExternalInput")
    eps_d = nc.dram_tensor("eps_nzq", [1, HWP], F32, kind="ExternalInput")

    with TileContext(nc) as tc, contextlib.ExitStack() as ctx:
        wp = ctx.enter_context(tc.tile_pool(name="wp", bufs=1))
        p1 = ctx.enter_context(tc.tile_pool(name="p1", bufs=1))
        p2 = ctx.enter_context(tc.tile_pool(name="p2", bufs=2))
        pm = ctx.enter_context(tc.tile_pool(name="pm", bufs=1, space="PSUM"))
        pT = ctx.enter_context(tc.tile_pool(name="pT", bufs=1, space="PSUM"))
        pAV = ctx.enter_context(tc.tile_pool(name="pAV", bufs=1, space="PSUM"))

        # ---- resident weights: two packed DMAs instead of 26 small ones ----
        wallb = wp.tile([128, PACKB_W], BF16, name="wallb", tag="wallb")
        wallf = wp.tile([128, PACKF_W], F32, name="wallf", tag="wallf")
        epst = wp.tile([1, HWP], F32, name="epst", tag="epst")
        nc.sync.dma_start(wallb[:, :], wpb_d[:, :])
        nc.sync.dma_start(wallf[:, :], wpf_d[:, :])
        nc.sync.dma_start(epst[:, :], eps_d[:, :])
        W = {"eps_nzq": [(epst, 1)]}
        for k, (which, width, entries) in PACK_LAYOUT.items():
            wall = wallb if which == "b" else wallf
            W[k] = [(wall[0:L, off : off + width], L) for (L, off) in entries]

        def w1(k):
            return W[k][0][0]

        pmctr = [0]

        def pm_tile():
            i = pmctr[0] % 5
            pmctr[0] += 1
            return pm.tile([128, 512], F32, name=f"pm{i}", tag=f"pm{i}")

        def alloc_row_tiles(pool, n_rows, width, tag, dtype=BF16, bufs=None):
            out = []
            for i, (o, L) in enumerate(chunks(n_rows)):
                out.append(
                    (
                        pool.tile(
                            [L, width], dtype, name=f"{tag}{i}", tag=f"{tag}{i}",
                            bufs=bufs,
                        ),
                        L,
                    )
                )
            return out

        def emit_mm(ps_sets, pairs, nsl):
            """ps_sets: [(ps, col_off, col_len)]; pairs: [(w_tiles, rhs_tiles)]
            where w_tiles/rhs_tiles are [(tile, rows)] lists zipped per chunk."""
            w_ = nsl.stop - nsl.start
            chunk_list = []
            for w_tiles, rhs_tiles in pairs:
                for (wt, wl), (rt, rl) in zip(w_tiles, rhs_tiles):
                    assert wl == rl, (wl, rl)
                    chunk_list.append((wt, rt, wl))
            for ps, c_off, c_len in ps_sets:
                for ki, (wt, rt, kl) in enumerate(chunk_list):
                    nc.tensor.matmul(
                        ps[:c_len, 0:w_],
                        lhsT=wt[:kl, c_off : c_off + c_len],
                        rhs=rt[:kl, nsl],
                        start=(ki == 0),
                        stop=(ki == len(chunk_list) - 1),
                    )

        def _ps_segs(g_off, dst_off, length, psums):
            for st, sr, dt_, dr, L in legal_segs(g_off, dst_off, length, src_sbuf=False):
                ps, m_off, m_len = psums[st]
                assert m_off == st * 128 and sr + L <= m_len
                yield ps, sr, dt_, dr, L, st

        def copy_h(psums, g_off, C_, h_tiles, nsl, bias_t):
            """h[c, nsl] = ps[g_off+c] + bias  (ACT identity, PSUM may shift)."""
            for ps, row, dt_, dr, L, m_idx in _ps_segs(g_off, 0, C_, psums):
                nc.scalar.activation(
                    h_tiles[dt_][0][dr : dr + L, nsl],
                    ps[row : row + L, 0:512],
                    AF.Identity,
                    bias=bias_t[row : row + L, m_idx : m_idx + 1],
                )

        def emit_elu(h_tiles, C_, st_tiles, inplace_h=True, width=HWP):
            """Single-exp elu streams from bf16 h (SBUF):
              pos rows [0,C):        st = max(1+h, e),  e = exp(-|h|)
              neg rows [PAD,PAD+C):  st = max(1-h, e)
            via t_p=1+h, t_n=2-t_p, q=max(t_p,t_n)=1+|h|, e=Exp(-q+1).
            Shared scratch tags sqq/see rotate across calls (p2 bufs=2).
            When inplace_h, h is overwritten with 1-h (h must be dead)."""
            qq = alloc_row_tiles(p2, C_, width, "sqq", bufs=1)
            ee = alloc_row_tiles(p2, C_, width, "see", bufs=1)
            # t_n = 1-h lives in h itself when h is dead, else in scratch.
            tn = (
                h_tiles
                if inplace_h
                else alloc_row_tiles(p2, C_, width, "stn", bufs=1)
            )
            for i, (ht, hl) in enumerate(h_tiles):
                st = st_tiles[i][0]
                nc.vector.tensor_scalar(st[0:hl, :], ht[:hl, :], 1.0, None, OP.add)
                nc.vector.tensor_scalar(
                    tn[i][0][:hl, :], st[0:hl, :], -1.0, 2.0, OP.mult, OP.add
                )
                nc.vector.tensor_tensor(
                    qq[i][0][:hl, :], st[0:hl, :], tn[i][0][:hl, :], op=OP.max
                )
                nc.scalar.activation(
                    ee[i][0][:hl, :], qq[i][0][:hl, :], AF.Exp, scale=-1.0, bias=1.0
                )
                nc.vector.tensor_tensor(
                    st[0:hl, :], st[0:hl, :], ee[i][0][:hl, :], op=OP.max
                )
            for st_i, sr, dt_, dr, L in legal_segs(0, PAD, C_):
                nc.vector.tensor_tensor(
                    st_tiles[dt_][0][dr : dr + L, :],
                    tn[st_i][0][sr : sr + L, :],
                    ee[st_i][0][sr : sr + L, :],
                    op=OP.max,
                )

        def emit_gate(psums, C_, bias_t, bga_t, out_tiles, nsl):
            """[ga05|pad|gb] psums -> out = (0.5ga + 0.5b_ga) * (tanh(...)+1).
            The DVE STT reads the 2-segment ga rows; tanh (ACT) takes the
            3-segment gb rows."""
            Tt = alloc_row_tiles(p1, C_, 512, "Tg")
            for ps, row, dt_, dr, L, m_idx in _ps_segs(PAD, 0, C_, psums):
                nc.scalar.activation(
                    Tt[dt_][0][dr : dr + L, 0:512],
                    ps[row : row + L, 0:512],
                    AF.Tanh,
                    bias=bias_t[row : row + L, m_idx : m_idx + 1],
                    scale=0.5,
                )
            for t_, tl in Tt:
                nc.vector.tensor_scalar(
                    t_[:tl, 0:512], t_[:tl, 0:512], 1.0, None, OP.add
                )
            for ps, row, dt_, dr, L, m_idx in _ps_segs(0, 0, C_, psums):
                nc.vector.scalar_tensor_tensor(
                    out_tiles[dt_][0][dr : dr + L, nsl],
                    ps[row : row + L, 0:512],
                    bga_t[dr : dr + L, dt_ : dt_ + 1],
                    Tt[dt_][0][dr : dr + L, 0:512],
                    OP.add,
                    OP.mult,
                )

        # ---------------- per-sample stages ----------------
        # Two samples are emitted interleaved stage-by-stage so each engine
        # fills its dependency bubbles with the sibling sample's work.

        def st_input(s, v):
            # bufs=4: C and Sc live to the end of the sample; four buffers
            # fully decouple the four in-flight samples' front ends.
            C0 = p2.tile([128, HWP], BF16, name="C0", tag="C0", bufs=4)
            C1 = p2.tile([41, HWP], BF16, name="C1", tag="C1", bufs=4)
            nc.sync.dma_start(C0[:, :], ul_d[s, 0:128, :])
            nc.sync.dma_start(C1[0:32, :], ul_d[s, 128:160, :])
            nc.sync.dma_start(C1[32:38, :], b_d[s, :, :])
            nc.sync.dma_start(C1[38:41, :], x_d[s, :, :])
            v["C_tiles"] = [(C0, 128), (C1, 41)]
            # input stream Sc [361 rows]
            Sc = alloc_row_tiles(p2, 361, HWP, "Sc", bufs=4)
            # zero pad rows once per pool buffer (elu rewrites 32:41)
            nc.gpsimd.memset(Sc[1][0][32:64, :], 0.0)
            emit_elu(v["C_tiles"], CK, Sc, inplace_h=False)
            v["Sc"] = Sc

        def st_nin1(s, v):
            # fused nin1 (k,v,q): rows [hk 0:169|pad|hv 192:361|pad|hq 384:550]
            hk = alloc_row_tiles(p2, CK, HWP, "hk")
            hv = alloc_row_tiles(p2, CK, HWP, "hv")
            hq = alloc_row_tiles(p2, CQ, HWP, "hq")
            for nco in range(0, HWP, 512):
                nsl = slice(nco, nco + 512)
                psums = []
                for m_off, m_len in chunks(550):
                    psums.append((pm_tile(), m_off, m_len))
                emit_mm(psums, [(W["wab_t"], v["Sc"])], nsl)
                copy_h(psums, 0, CK, hk, nsl, w1("bab"))
                copy_h(psums, PAD, CK, hv, nsl, w1("bab"))
                copy_h(psums, 2 * PAD, CQ, hq, nsl, w1("bab"))
            v["hk"], v["hv"], v["hq"] = hk, hv, hq

        def make_grn(key, h_key, C_, wo_key, bo_key, bg_key):
            def st_grn(s, v):
                h_t = v[h_key]
                St = alloc_row_tiles(p2, PAD + C_, HWP, f"S{key}")
                if s < 2:
                    nc.gpsimd.memset(St[1][0][32:64, :], 0.0)
                emit_elu(h_t, C_, St)
                G = h_t  # h dead after elu; reuse its tiles for the gate out
                for nco in range(0, HWP, 512):
                    nsl = slice(nco, nco + 512)
                    psums = []
                    for m_off, m_len in chunks(PAD + C_):
                        psums.append((pm_tile(), m_off, m_len))
                    emit_mm(psums, [(W[wo_key], St)], nsl)
                    emit_gate(psums, C_, w1(bo_key), w1(bg_key), G, nsl)
                v[f"G{key}"] = G

            return st_grn

        def st_proj(s, v):
            # K/Q/V projection (+ folded C residual)
            K_sb = p2.tile([KD, HWP], BF16, name="Ksb", tag="Ksb")
            Q_sb = p2.tile([KD, HWP], BF16, name="Qsb", tag="Qsb")
            V_sb = p2.tile([VD, HWP], BF16, name="Vsb", tag="Vsb")
            for nco in range(0, HWP, 512):
                nsl = slice(nco, nco + 512)
                ps0, ps1 = pm_tile(), pm_tile()
                emit_mm(
                    [(ps0, 0, 48)],
                    [
                        (W["pjk"], v["Gk"]),
                        (W["pjq"], v["Gq"]),
                        (W["pjc"], v["C_tiles"]),
                    ],
                    nsl,
                )
                emit_mm(
                    [(ps1, 0, 80)],
                    [(W["pjv"], v["Gv"]), (W["pjv"], v["C_tiles"])],
                    nsl,
                )
                nc.scalar.activation(
                    K_sb[0:KD, nsl], ps0[0:KD, 0:512], AF.Identity,
                    bias=w1("njb")[0:KD, 0:1],
                )
                nc.scalar.activation(
                    Q_sb[0:KD, nsl], ps0[32:48, 0:512], AF.Identity,
                    bias=w1("njb")[32:48, 0:1],
                )
                nc.scalar.activation(
                    V_sb[0:VD, nsl], ps1[0:VD, 0:512], AF.Identity,
                    bias=w1("njb")[0:VD, 1:2],
                )
            v["K_sb"], v["Q_sb"], v["V_sb"] = K_sb, Q_sb, V_sb

        def st_attn(s, v):
            # E = exp(K^T Q) per k-tile, strict-causal mask on diag block
            E_att = []
            for kt in range(8):
                h0 = kt // 4
                qstart = 512 * h0
                ew = HWP - qstart
                et = p1.tile([128, ew], BF16, name=f"Eatt{kt}", tag=f"Eatt{kt}")
                E_att.append((et, qstart))
                zpad = (kt % 4) * 128
                spans = [(128 * kt, 512 * (h0 + 1))]
                if h0 == 0:
                    spans.append((512, 1024))
                for ga, gb_ in spans:
                    ps = pm_tile()
                    w_ = gb_ - ga
                    nc.tensor.matmul(
                        ps[:, 0:w_],
                        lhsT=v["K_sb"][0:KD, kt * 128 : (kt + 1) * 128],
                        rhs=v["Q_sb"][0:KD, ga:gb_],
                        start=True,
                        stop=True,
                    )
                    nc.scalar.activation(
                        et[:, ga - qstart : gb_ - qstart], ps[:, 0:w_], AF.Exp
                    )
                nc.gpsimd.tensor_tensor(
                    et[:, zpad : zpad + 128],
                    et[:, zpad : zpad + 128],
                    w1("masks")[:, 0:128],
                    op=OP.mult,
                )
            v["E_att"] = E_att
            # V^T (+ ones row for softmax row sums) via PE transpose
            VT = []
            for pc in range(8):
                pst = pT.tile([128, 512], BF16, name="St", tag="St")
                nc.tensor.transpose(
                    pst[:, 0:80],
                    v["V_sb"][:VD, pc * 128 : (pc + 1) * 128],
                    w1("ident80")[:80, :80],
                )
                vt = p1.tile([128, 97], BF16, name=f"VT{pc}", tag=f"VT{pc}")
                nc.vector.tensor_copy(vt[:, 0:80], pst[:, 0:80])
                if s < 1:  # cols 80:97 (zeros + ones row) never change
                    nc.gpsimd.memset(vt[:, 80:96], 0.0)
                    nc.gpsimd.memset(vt[:, 96:97], 1.0)
                VT.append(vt)
            v["VT"] = VT

            # AV accumulate per 512-half; row 96 = sum_k E (softmax denom).
            # Separate banks per half; rhs starts at the k-tile's first
            # nonzero column (cols left of it are never written or read).
            att = p1.tile([VD, HWP], BF16, name="att", tag="att")
            for qc in range(2):
                qsl = slice(qc * 512, (qc + 1) * 512)
                pav = pAV.tile([97, 512], F32, name=f"AV{qc}", tag=f"AV{qc}")
                kts = [kt for kt in range(8) if 128 * kt < (qc + 1) * 512]
                for i, kt in enumerate(kts):
                    et, qstart = v["E_att"][kt]
                    zpad = (kt % 4) * 128
                    c0 = qc * 512 - qstart
                    lo = max(c0, zpad)
                    nc.tensor.matmul(
                        pav[:97, lo - c0 : 512],
                        lhsT=v["VT"][kt][:, 0:97],
                        rhs=et[:, lo : c0 + 512],
                        start=(i == 0),
                        stop=(i == len(kts) - 1),
                        skip_group_check=(lo != c0),
                    )
                # att = AV[0:80] / ((1+eps)*R + eps*(1024-q))
                den_t = p1.tile([1, 512], F32, name=f"den{qc}", tag=f"den{qc}")
                nc.vector.scalar_tensor_tensor(
                    den_t[0:1, :], pav[96:97, 0:512], 1.0 + EPS,
                    w1("eps_nzq")[0:1, qsl], OP.mult, OP.add,
                )
                nc.vector.reciprocal_approx_fast(den_t[0:1, :], den_t[0:1, :])
                attb = p1.tile([VD, 512], F32, name=f"attb{qc}", tag=f"attb{qc}")
                nc.gpsimd.partition_broadcast(attb[:VD, :], den_t[0:1, :])
                nc.vector.tensor_tensor(
                    att[:VD, qsl], pav[0:VD, 0:512], attb[:VD, :], op=OP.mult
                )

            # att stream Sa: pos/neg tiles [80]; single-exp elu, att -> 1-att
            # (bufs=2: Sa crosses the stage boundary into st_out1 under the
            # two-sample interleave)
            Sa_p = p1.tile([VD, HWP], BF16, name="Sap", tag="Sap", bufs=2)
            Sa_n = p1.tile([VD, HWP], BF16, name="San", tag="San", bufs=2)
            aha = p1.tile([VD, HWP], BF16, name="aha", tag="aha")
            eea = p1.tile([VD, HWP], BF16, name="eea", tag="eea")
            nc.vector.tensor_scalar(Sa_p[:VD, :], att[:VD, :], 1.0, None, OP.add)
            nc.vector.tensor_scalar(
                att[:VD, :], Sa_p[:VD, :], -1.0, 2.0, OP.mult, OP.add
            )
            nc.vector.tensor_tensor(
                aha[:VD, :], Sa_p[:VD, :], att[:VD, :], op=OP.max
            )
            nc.scalar.activation(
                eea[:VD, :], aha[:VD, :], AF.Exp, scale=-1.0, bias=1.0
            )
            nc.vector.tensor_tensor(
                Sa_p[:VD, :], Sa_p[:VD, :], eea[:VD, :], op=OP.max
            )
            nc.vector.tensor_tensor(Sa_n[:VD, :], att[:VD, :], eea[:VD, :], op=OP.max)
            v["Sa_p"], v["Sa_n"] = Sa_p, Sa_n

        def st_out1(s, v):
            # bufs=2: ho crosses into st_out2 under the two-sample interleave
            ho = alloc_row_tiles(p1, NF, HWP, "ho", bufs=2)
            for nco in range(0, HWP, 512):
                nsl = slice(nco, nco + 512)
                psums = []
                for m_off, m_len in chunks(NF):
                    psums.append((pm_tile(), m_off, m_len))
                emit_mm(
                    psums,
                    [
                        (W["wo1a_t"], v["Sc"]),
                        (W["wo1bp"], [(v["Sa_p"], VD)]),
                        (W["wo1bn"], [(v["Sa_n"], VD)]),
                    ],
                    nsl,
                )
                copy_h(psums, 0, NF, ho, nsl, w1("bo1"))
            v["ho"] = ho

        def st_out2(s, v):
            So = alloc_row_tiles(p1, PAD + NF, HWP, "So")
            if s < 1:
                nc.gpsimd.memset(So[1][0][32:64, :], 0.0)
            emit_elu(v["ho"], NF, So)
            Opre = alloc_row_tiles(p1, NF, HWP, "Opre")
            O0 = p1.tile([128, HWP], BF16, name="O0", tag="O0")
            O1 = p1.tile([32, HWP], BF16, name="O1", tag="O1")
            for nco in range(0, HWP, 512):
                nsl = slice(nco, nco + 512)
                psums = []
                for m_off, m_len in chunks(PAD + NF):
                    psums.append((pm_tile(), m_off, m_len))
                emit_mm(psums, [(W["woo_t"], So)], nsl)
                emit_gate(psums, NF, w1("boo"), w1("bgo"), Opre, nsl)
            C0, C1 = v["C_tiles"][0][0], v["C_tiles"][1][0]
            nc.gpsimd.tensor_tensor(O0[:, :], Opre[0][0][:, :], C0[:, :], op=OP.add)
            nc.gpsimd.tensor_tensor(
                O1[:32, :], Opre[1][0][:32, :], C1[0:32, :], op=OP.add
            )
            nc.gpsimd.dma_start(out_d[s, 0:128, :], O0[:, :])
            nc.gpsimd.dma_start(out_d[s, 128:160, :], O1[:32, :])

        stages = [
            st_input,
            st_nin1,
            make_grn("k", "hk", CK, "wok_t", "bok", "bgk"),
            make_grn("q", "hq", CQ, "woq_t", "boq", "bgq"),
            make_grn("v", "hv", CK, "wov_t", "bov", "bgv"),
            st_proj,
            st_attn,
            st_out1,
            st_out2,
        ]
        assert ns % 2 == 0
        for pair in range(ns // 2):
            va, vb = {}, {}
            for stage in stages:
                stage(2 * pair, va)
                stage(2 * pair + 1, vb)

    nc.compile()
    return nc


_NC_CACHE = {}


def _get_nc():
    if "nc" not in _NC_CACHE:
        _NC_CACHE["nc"] = build_nc()
    return _NC_CACHE["nc"]


def make_in_maps(inputs):
    inp = {
        k: np.ascontiguousarray(np.asarray(v), dtype=np.float32)
        for k, v in inputs.items()
    }
    p = prep_weights(inp)
    for k, sshape in WSHAPES.items():
        assert p[k].shape == sshape, (k, p[k].shape, sshape)

    x = inp["x"].reshape(N, XD, HWP).astype(BF)
    ul = inp["ul"].reshape(N, NF, HWP).astype(BF)
    b = inp["b"].reshape(N, 2 * XD, HWP).astype(BF)
    wpb, wpf = pack_weights(p)

    in_maps = []
    for c in range(NCORES):
        sl = slice(c * NS, (c + 1) * NS)
        m = {
            "x": x[sl], "ul": ul[sl], "b": b[sl],
            "wpb": wpb, "wpf": wpf, "eps_nzq": p["eps_nzq"],
        }
        in_maps.append(m)
    return in_maps


def kernel(**inputs):
    in_maps = make_in_maps(inputs)
    nc = _get_nc()
    res = run_bass_kernel_spmd(nc, in_maps, core_ids=list(range(NCORES)))
    out = np.concatenate([r["out"] for r in res.results], axis=0)
    return out.reshape(N, NF, 32, 32)


if __name__ == "__main__":
    import reference as R

    inputs = {k: np.asarray(v) for k, v in R.setup_inputs().items()}
    got = kernel(**inputs)
    exp = np.asarray(R.reference(**R.setup_inputs()))
    err = np.abs(got - exp)
    print("max abs err:", err.max(), "rel:", err.max() / np.abs(exp).max())



# revision 42
# speedup vs baseline: 1.3013x; 1.3013x over previous
"""Trainium2 Bass kernel for nn_AttentionBlock (causal attention block), v2.

Self-contained: takes FULL inputs (batch 32), shards batch over 8 NeuronCores
(4 samples/core, pure data parallel), runs a Bass/Tile kernel per core, and
gathers the full [32, 160, 32, 32] output.

v2 design (vs the fp32r baseline):
- bf16 matmuls and bf16 SBUF data everywhere (rel-err budget 2e-2 allows it):
  PE runs at 1 cycle/row instead of fp32r's ~3, and DVE element-wise ops get
  the 16-bit 2x mode.
- no identity-copy of nin1 outputs into fp32: h is copied PSUM->SBUF bf16 once
  (ACT identity + bias, PSUM sources may shift partitions), then all elu math
  runs 1024-wide on bf16 SBUF tiles.
- elu decomposition per sign, from m = min(h,0), rp = relu(h):
    stream_pos = exp(m) + rp        stream_neg = exp(-rp) - m
  (exp on ScalarE; min/max maps + adds on DVE/GPSIMD per ENG table).
- gate: nin2 out layout [gb | pad | 0.5*ga]; T = tanh(0.5*gb + 0.5*b_gb) + 1;
  G = (0.5*ga + 0.5*b_ga) * T.  The grn residual (+C) for the k/q/v GRNs is
  folded into the K/Q/V projection matmuls (proj(G) + proj(C)); the output
  GRN adds ul explicitly.
- K/Q/V projections run as two accumulated PSUM sets ([K|pad|Q] 48 rows, [V]
  80 rows) sharing the C-residual matmuls.
- attention identical in structure to baseline (S^T per k-tile, exp without
  max-subtraction, ones-row in V^T for free softmax denominators), in bf16.
"""

import sys

sys.path.insert(0, "/opt/trn_rl_repo")

import contextlib

import ml_dtypes
import numpy as np

import concourse.bacc as bacc
import concourse.mybir as mybir
from concourse.bass_utils import run_bass_kernel_spmd
from concourse.tile import TileContext

F32 = mybir.dt.float32
BF16 = mybir.dt.bfloat16
AF = mybir.ActivationFunctionType
OP = mybir.AluOpType
BF = ml_dtypes.bfloat16

N, XD, NF = 32, 3, 160
KD, VD = 16, 80
CK, CQ = 169, 166
HWP = 1024
NS = 4  # samples per core
NCORES = 8
EPS = 1e-7
PAD = 192  # elu- stream offset

# v3: single-exp elu streams.  From e = exp(-|h|):
#   pos = elu(h)+1  = max(h+1, e)   (exp(x) >= 1+x, equality at 0)
#   neg = elu(-h)+1 = max(1-h, e)
# One ACT pass per elu instead of two; DVE work uses tensor_scalar (4x mode)
# and tensor_tensor (2x) instead of scalar_tensor_tensor (no fast mode).


def chunks(total, step=128):
    return [(o, min(step, total - o)) for o in range(0, total, step)]


_PLIMIT = {0: 128, 32: 32, 64: 64, 96: 32}


def legal_segs(src_off, dst_off, length, src_sbuf=True):
    """Split a row-range copy into SBUF-legal pieces (windows at 0/32/64/96).
    PSUM sources are exempt.  Yields (src_tile, src_row, dst_tile, dst_row, L).
    """
    done = 0
    while done < length:
        s, d = src_off + done, dst_off + done
        sb, db = s % 128, d % 128
        L = min(_PLIMIT[db], 128 - db, length - done)
        if src_sbuf:
            L = min(L, _PLIMIT[sb], 128 - sb)
        else:
            L = min(L, 128 - sb)
        yield (s // 128, sb, d // 128, db, L)
        done += L


# ---------------------------------------------------------------- host prep --


def bias_chunked(bias):
    nm = (len(bias) + 127) // 128
    t = np.zeros((128, nm), np.float32)
    for m in range(nm):
        seg = bias[128 * m : 128 * (m + 1)]
        t[: len(seg), m] = seg
    return t


def prep_weights(inp):
    """Numpy prep: permutations, stream packing, bias folds, 0.5 gate scaling.

    Channel order 'cb' = [ul(160), b(6), x(3)].  Streams (matmul rhs rows):
    [elu+ (C) | pad->192 | elu- (C)].  nin2 out layout [gb | pad | 0.5*ga].
    Streams hold elu(x)+1, so each consumer's bias folds -W.sum(1).
    """
    p = {}
    perm_k = np.array(list(range(3, 169)) + list(range(0, 3)))
    perm_q = np.arange(166)

    def stream_cols(Wi, perm):
        C = Wi.shape[1] // 2
        W1, W2 = Wi[:, :C][:, perm], Wi[:, C:][:, perm]
        out = np.zeros((Wi.shape[0], 361), np.float32)
        out[:, : W1.shape[1]] = W1
        out[:, PAD : PAD + W2.shape[1]] = W2
        return out, Wi.sum(1)

    kW, kfold = stream_cols(inp["gkWi"], perm_k)
    vW, vfold = stream_cols(inp["gvWi"], perm_k)
    qW, qfold = stream_cols(inp["gqWi"], perm_q)
    Wab = np.zeros((550, 361), np.float32)
    Wab[0:169] = kW
    Wab[192:361] = vW
    Wab[384:550] = qW
    bab = np.zeros(550, np.float32)
    bab[0:169] = inp["gkbi"] - kfold
    bab[192:361] = inp["gvbi"] - vfold
    bab[384:550] = inp["gqbi"] - qfold
    p["wab_t"] = Wab.T  # [361, 550]
    p["bab"] = bias_chunked(bab)

    def inner_w(Wo, bo, out_perm):
        """nin2 lhsT [stream rows, out rows] with out layout [ga05|pad|gb]:
        ga at rows [0,C) so the DVE gate STT drains in 2 aligned segments;
        gb (tanh, on ACT) takes the fragmented rows [PAD, PAD+C).
        bias tile holds 0.5*b_gb at gb rows (for the tanh AP bias)."""
        C = Wo.shape[1] // 2
        W1, W2 = Wo[:, :C], Wo[:, C:]
        bias = bo - (W1.sum(1) + W2.sum(1))
        Wfull = np.concatenate([W1, W2], axis=1)  # [2C out, 2C in]
        gb_w = Wfull[C + out_perm]
        ga_w = Wfull[out_perm] * 0.5
        n = PAD + C
        Ws = np.zeros((n, n), np.float32)
        for rows, w_ in ((slice(0, C), ga_w), (slice(PAD, n), gb_w)):
            Ws[rows, 0:C] = w_[:, 0:C]
            Ws[rows, PAD : PAD + C] = w_[:, C : 2 * C]
        bs = np.zeros(n, np.float32)
        bs[PAD:n] = 0.5 * bias[C + out_perm]
        # ga-part bias indexed by destination row (channel), for the gate STT
        bga = np.zeros(256, np.float32)
        bga[0:C] = 0.5 * bias[out_perm]
        return Ws.T, bias_chunked(bs), bias_chunked(bga)

    p["wok_t"], p["bok"], p["bgk"] = inner_w(inp["gkWo"], inp["gkbo"], perm_k)
    p["woq_t"], p["boq"], p["bgq"] = inner_w(inp["gqWo"], inp["gqbo"], perm_q)
    p["wov_t"], p["bov"], p["bgv"] = inner_w(inp["gvWo"], inp["gvbo"], perm_k)
    p["woo_t"], p["boo"], p["bgo"] = inner_w(inp["goWo"], inp["gobo"], np.arange(NF))

    # K/Q/V projections with folded +C residual.
    nk = inp["nkW"][:, perm_k]  # [16, 169]
    nq = inp["nqW"][:, perm_q]  # [16, 166]
    nv = inp["nvW"][:, perm_k]  # [80, 169]
    pjk = np.zeros((CK, 48), np.float32)
    pjk[:, 0:16] = nk.T
    pjq = np.zeros((CQ, 48), np.float32)
    pjq[:, 32:48] = nq.T
    pjc = np.zeros((CK, 48), np.float32)
    pjc[:, 0:16] = nk.T
    pjc[0:CQ, 32:48] = nq.T
    p["pjk"], p["pjq"], p["pjc"] = pjk, pjq, pjc
    p["pjv"] = np.ascontiguousarray(nv.T)  # used for both G_v and C chunks
    njb = np.zeros((128, 2), np.float32)
    njb[0:16, 0] = inp["nkb"]
    njb[32:48, 0] = inp["nqb"]
    njb[0:80, 1] = inp["nvb"]
    p["njb"] = njb

    # grn_out nin1: Sc(ul rows) + Sa(att stream) -> h_o [160]
    W1, W2 = inp["goWi"][:, :NF], inp["goWi"][:, NF:]
    wa = np.zeros((NF, 361), np.float32)
    wa[:, 0:NF] = W1
    wa[:, PAD : PAD + NF] = W2
    p["wo1a_t"] = wa.T  # [361, 160]
    p["wo1bp"] = np.ascontiguousarray(inp["goWs"][:, :VD].T)  # [80, 160]
    p["wo1bn"] = np.ascontiguousarray(inp["goWs"][:, VD:].T)
    p["bo1"] = bias_chunked(
        (inp["gobi"] + inp["gobs"] - inp["goWi"].sum(1) - inp["goWs"].sum(1)).astype(
            np.float32
        )
    )

    pp = np.arange(128)[:, None]
    ff = np.arange(128)[None, :]
    p["masks"] = (ff > pp).astype(np.float32)
    p["eps_nzq"] = (EPS * (HWP - np.arange(HWP, dtype=np.float32)))[None, :]
    p["ident80"] = np.eye(80, dtype=np.float32)

    out = {}
    for k, v in p.items():
        dt = np.float32 if k in F32_W else BF
        out[k] = np.ascontiguousarray(np.asarray(v, dtype=np.float32).astype(dt))
    return out


WSHAPES = {
    "wab_t": (361, 550),
    "wok_t": (361, 361),
    "woq_t": (358, 358),
    "wov_t": (361, 361),
    "woo_t": (352, 352),
    "wo1a_t": (361, 160),
    "wo1bp": (80, 160),
    "wo1bn": (80, 160),
    "pjk": (169, 48),
    "pjq": (166, 48),
    "pjc": (169, 48),
    "pjv": (169, 80),
    "bab": (128, 5),
    "bok": (128, 3),
    "bgk": (128, 2),
    "bgq": (128, 2),
    "bgv": (128, 2),
    "bgo": (128, 2),
    "boq": (128, 3),
    "bov": (128, 3),
    "boo": (128, 3),
    "bo1": (128, 2),
    "njb": (128, 2),
    "masks": (128, 128),
    "eps_nzq": (1, HWP),
    "ident80": (80, 80),
}
F32_W = {"bab", "bok", "boq", "bov", "boo", "bo1", "njb", "eps_nzq", "bgk", "bgq", "bgv", "bgo"}


def _pack_layout():
    """Column layout packing every weight chunk side-by-side into one bf16
    and one f32 [128, *] tensor (single DMA each).  eps_nzq stays separate
    (it is [1, 1024])."""
    layout = {}
    offs = {"b": 0, "f": 0}
    for k, (rows, width) in WSHAPES.items():
        if k == "eps_nzq":
            continue
        which = "f" if k in F32_W else "b"
        entries = []
        for o, L in chunks(rows):
            entries.append((L, offs[which]))
            offs[which] += width
        layout[k] = (which, width, entries)
    return layout, offs["b"], offs["f"]


PACK_LAYOUT, PACKB_W, PACKF_W = _pack_layout()


def pack_weights(p):
    wpb = np.zeros((128, PACKB_W), BF)
    wpf = np.zeros((128, PACKF_W), np.float32)
    for k, (which, width, entries) in PACK_LAYOUT.items():
        arr = p[k]
        dst = wpb if which == "b" else wpf
        for (o, L), (L2, off) in zip(chunks(WSHAPES[k][0]), entries):
            dst[0:L, off : off + width] = arr[o : o + L, :]
    return wpb, wpf


def build_nc(ns=NS):
    nc = bacc.Bacc("TRN2", target_bir_lowering=False, debug=False)

    x_d = nc.dram_tensor("x", [ns, XD, HWP], BF16, kind="ExternalInput")
    ul_d = nc.dram_tensor("ul", [ns, NF, HWP], BF16, kind="ExternalInput")
    b_d = nc.dram_tensor("b", [ns, 2 * XD, HWP], BF16, kind="ExternalInput")
    out_d = nc.dram_tensor("out", [ns, NF, HWP], F32, kind="ExternalOutput")
    wpb_d = nc.dram_tensor("wpb", [128, PACKB_W], BF16, kind="ExternalInput")
    wpf_d = nc.dram_tensor("wpf", [128, PACKF_W], F32, kind="ExternalInput")
    eps_d = nc.dram_tensor("eps_nzq", [1, HWP], F32, kind="ExternalInput")

    with TileContext(nc) as tc, contextlib.ExitStack() as ctx:
        wp = ctx.enter_context(tc.tile_pool(name="wp", bufs=1))
        p1 = ctx.enter_context(tc.tile_pool(name="p1", bufs=1))
        p2 = ctx.enter_context(tc.tile_pool(name="p2", bufs=2))
        pm = ctx.enter_context(tc.tile_pool(name="pm", bufs=1, space="PSUM"))
        pT = ctx.enter_context(tc.tile_pool(name="pT", bufs=1, space="PSUM"))
        pAV = ctx.enter_context(tc.tile_pool(name="pAV", bufs=1, space="PSUM"))

        # ---- resident weights: two packed DMAs instead of 26 small ones ----
        wallb = wp.tile([128, PACKB_W], BF16, name="wallb", tag="wallb")
        wallf = wp.tile([128, PACKF_W], F32, name="wallf", tag="wallf")
        epst = wp.tile([1, HWP], F32, name="epst", tag="epst")
        nc.sync.dma_start(wallb[:, :], wpb_d[:, :])
        nc.sync.dma_start(wallf[:, :], wpf_d[:, :])
        nc.sync.dma_start(epst[:, :], eps_d[:, :])
        W = {"eps_nzq": [(epst, 1)]}
        for k, (which, width, entries) in PACK_LAYOUT.items():
            wall = wallb if which == "b" else wallf
            W[k] = [(wall[0:L, off : off + width], L) for (L, off) in entries]

        def w1(k):
            return W[k][0][0]

        pmctr = [0]

        def pm_tile():
            i = pmctr[0] % 5
            pmctr[0] += 1
            return pm.tile([128, 512], F32, name=f"pm{i}", tag=f"pm{i}")

        def alloc_row_tiles(pool, n_rows, width, tag, dtype=BF16, bufs=None):
            out = []
            for i, (o, L) in enumerate(chunks(n_rows)):
                out.append(
                    (
                        pool.tile(
                            [L, width], dtype, name=f"{tag}{i}", tag=f"{tag}{i}",
                            bufs=bufs,
                        ),
                        L,
                    )
                )
            return out

        def emit_mm(ps_sets, pairs, nsl):
            """ps_sets: [(ps, col_off, col_len)]; pairs: [(w_tiles, rhs_tiles)]
            where w_tiles/rhs_tiles are [(tile, rows)] lists zipped per chunk."""
            w_ = nsl.stop - nsl.start
            chunk_list = []
            for w_tiles, rhs_tiles in pairs:
                for (wt, wl), (rt, rl) in zip(w_tiles, rhs_tiles):
                    assert wl == rl, (wl, rl)
                    chunk_list.append((wt, rt, wl))
            for ps, c_off, c_len in ps_sets:
                for ki, (wt, rt, kl) in enumerate(chunk_list):
                    nc.tensor.matmul(
                        ps[:c_len, 0:w_],
                        lhsT=wt[:kl, c_off : c_off + c_len],
                        rhs=rt[:kl, nsl],
                        start=(ki == 0),
                        stop=(ki == len(chunk_list) - 1),
                    )

        def _ps_segs(g_off, dst_off, length, psums):
            for st, sr, dt_, dr, L in legal_segs(g_off, dst_off, length, src_sbuf=False):
                ps, m_off, m_len = psums[st]
                assert m_off == st * 128 and sr + L <= m_len
                yield ps, sr, dt_, dr, L, st

        def copy_h(psums, g_off, C_, h_tiles, nsl, bias_t):
            """h[c, nsl] = ps[g_off+c] + bias  (ACT identity, PSUM may shift)."""
            for ps, row, dt_, dr, L, m_idx in _ps_segs(g_off, 0, C_, psums):
                nc.scalar.activation(
                    h_tiles[dt_][0][dr : dr + L, nsl],
                    ps[row : row + L, 0:512],
                    AF.Identity,
                    bias=bias_t[row : row + L, m_idx : m_idx + 1],
                )

        def emit_elu(h_tiles, C_, st_tiles, inplace_h=True, width=HWP):
            """Single-exp elu streams from bf16 h (SBUF):
              pos rows [0,C):        st = max(1+h, e),  e = exp(-|h|)
              neg rows [PAD,PAD+C):  st = max(1-h, e)
            via t_p=1+h, t_n=2-t_p, q=max(t_p,t_n)=1+|h|, e=Exp(-q+1).
            Shared scratch tags sqq/see rotate across calls (p2 bufs=2).
            When inplace_h, h is overwritten with 1-h (h must be dead)."""
            qq = alloc_row_tiles(p2, C_, width, "sqq")
            ee = alloc_row_tiles(p2, C_, width, "see")
            # t_n = 1-h lives in h itself when h is dead, else in scratch.
            tn = h_tiles if inplace_h else alloc_row_tiles(p2, C_, width, "stn")
            for i, (ht, hl) in enumerate(h_tiles):
                st = st_tiles[i][0]
                nc.vector.tensor_scalar(st[0:hl, :], ht[:hl, :], 1.0, None, OP.add)
                nc.vector.tensor_scalar(
                    tn[i][0][:hl, :], st[0:hl, :], -1.0, 2.0, OP.mult, OP.add
                )
                nc.vector.tensor_tensor(
                    qq[i][0][:hl, :], st[0:hl, :], tn[i][0][:hl, :], op=OP.max
                )
                nc.scalar.activation(
                    ee[i][0][:hl, :], qq[i][0][:hl, :], AF.Exp, scale=-1.0, bias=1.0
                )
                nc.vector.tensor_tensor(
                    st[0:hl, :], st[0:hl, :], ee[i][0][:hl, :], op=OP.max
                )
            for st_i, sr, dt_, dr, L in legal_segs(0, PAD, C_):
                nc.vector.tensor_tensor(
                    st_tiles[dt_][0][dr : dr + L, :],
                    tn[st_i][0][sr : sr + L, :],
                    ee[st_i][0][sr : sr + L, :],
                    op=OP.max,
                )

        def emit_gate(psums, C_, bias_t, bga_t, out_tiles, nsl):
            """[ga05|pad|gb] psums -> out = (0.5ga + 0.5b_ga) * (tanh(...)+1).
            The DVE STT reads the 2-segment ga rows; tanh (ACT) takes the
            3-segment gb rows."""
            Tt = alloc_row_tiles(p1, C_, 512, "Tg")
            for ps, row, dt_, dr, L, m_idx in _ps_segs(PAD, 0, C_, psums):
                nc.scalar.activation(
                    Tt[dt_][0][dr : dr + L, 0:512],
                    ps[row : row + L, 0:512],
                    AF.Tanh,
                    bias=bias_t[row : row + L, m_idx : m_idx + 1],
                    scale=0.5,
                )
            for t_, tl in Tt:
                nc.vector.tensor_scalar(
                    t_[:tl, 0:512], t_[:tl, 0:512], 1.0, None, OP.add
                )
            for ps, row, dt_, dr, L, m_idx in _ps_segs(0, 0, C_, psums):
                nc.vector.scalar_tensor_tensor(
                    out_tiles[dt_][0][dr : dr + L, nsl],
                    ps[row : row + L, 0:512],
                    bga_t[dr : dr + L, dt_ : dt_ + 1],
                    Tt[dt_][0][dr : dr + L, 0:512],
                    OP.add,
                    OP.mult,
                )

        # ---------------- per-sample stages ----------------
        # Two samples are emitted interleaved stage-by-stage so each engine
        # fills its dependency bubbles with the sibling sample's work.

        def st_input(s, v):
            # bufs=3: C and Sc live to the end of the sample; a third buffer
            # lets the next pair's front end start during this pair's tail.
            C0 = p2.tile([128, HWP], BF16, name="C0", tag="C0", bufs=3)
            C1 = p2.tile([41, HWP], BF16, name="C1", tag="C1", bufs=3)
            nc.sync.dma_start(C0[:, :], ul_d[s, 0:128, :])
            nc.sync.dma_start(C1[0:32, :], ul_d[s, 128:160, :])
            nc.sync.dma_start(C1[32:38, :], b_d[s, :, :])
            nc.sync.dma_start(C1[38:41, :], x_d[s, :, :])
            v["C_tiles"] = [(C0, 128), (C1, 41)]
            # input stream Sc [361 rows]
            Sc = alloc_row_tiles(p2, 361, HWP, "Sc", bufs=3)
            if s < 3:  # zero pad rows once per pool buffer (elu rewrites 32:41)
                nc.gpsimd.memset(Sc[1][0][32:64, :], 0.0)
            emit_elu(v["C_tiles"], CK, Sc, inplace_h=False)
            v["Sc"] = Sc

        def st_nin1(s, v):
            # fused nin1 (k,v,q): rows [hk 0:169|pad|hv 192:361|pad|hq 384:550]
            hk = alloc_row_tiles(p2, CK, HWP, "hk")
            hv = alloc_row_tiles(p2, CK, HWP, "hv")
            hq = alloc_row_tiles(p2, CQ, HWP, "hq")
            for nco in range(0, HWP, 512):
                nsl = slice(nco, nco + 512)
                psums = []
                for m_off, m_len in chunks(550):
                    psums.append((pm_tile(), m_off, m_len))
                emit_mm(psums, [(W["wab_t"], v["Sc"])], nsl)
                copy_h(psums, 0, CK, hk, nsl, w1("bab"))
                copy_h(psums, PAD, CK, hv, nsl, w1("bab"))
                copy_h(psums, 2 * PAD, CQ, hq, nsl, w1("bab"))
            v["hk"], v["hv"], v["hq"] = hk, hv, hq

        def make_grn(key, h_key, C_, wo_key, bo_key, bg_key):
            def st_grn(s, v):
                h_t = v[h_key]
                St = alloc_row_tiles(p2, PAD + C_, HWP, f"S{key}")
                if s < 2:
                    nc.gpsimd.memset(St[1][0][32:64, :], 0.0)
                emit_elu(h_t, C_, St)
                G = h_t  # h dead after elu; reuse its tiles for the gate out
                for nco in range(0, HWP, 512):
                    nsl = slice(nco, nco + 512)
                    psums = []
                    for m_off, m_len in chunks(PAD + C_):
                        psums.append((pm_tile(), m_off, m_len))
                    emit_mm(psums, [(W[wo_key], St)], nsl)
                    emit_gate(psums, C_, w1(bo_key), w1(bg_key), G, nsl)
                v[f"G{key}"] = G

            return st_grn

        def st_proj(s, v):
            # K/Q/V projection (+ folded C residual)
            K_sb = p2.tile([KD, HWP], BF16, name="Ksb", tag="Ksb")
            Q_sb = p2.tile([KD, HWP], BF16, name="Qsb", tag="Qsb")
            V_sb = p2.tile([VD, HWP], BF16, name="Vsb", tag="Vsb")
            for nco in range(0, HWP, 512):
                nsl = slice(nco, nco + 512)
                ps0, ps1 = pm_tile(), pm_tile()
                emit_mm(
                    [(ps0, 0, 48)],
                    [
                        (W["pjk"], v["Gk"]),
                        (W["pjq"], v["Gq"]),
                        (W["pjc"], v["C_tiles"]),
                    ],
                    nsl,
                )
                emit_mm(
                    [(ps1, 0, 80)],
                    [(W["pjv"], v["Gv"]), (W["pjv"], v["C_tiles"])],
                    nsl,
                )
                nc.scalar.activation(
                    K_sb[0:KD, nsl], ps0[0:KD, 0:512], AF.Identity,
                    bias=w1("njb")[0:KD, 0:1],
                )
                nc.scalar.activation(
                    Q_sb[0:KD, nsl], ps0[32:48, 0:512], AF.Identity,
                    bias=w1("njb")[32:48, 0:1],
                )
                nc.scalar.activation(
                    V_sb[0:VD, nsl], ps1[0:VD, 0:512], AF.Identity,
                    bias=w1("njb")[0:VD, 1:2],
                )
            v["K_sb"], v["Q_sb"], v["V_sb"] = K_sb, Q_sb, V_sb

        def st_attn(s, v):
            # E = exp(K^T Q) per k-tile, strict-causal mask on diag block
            E_att = []
            for kt in range(8):
                h0 = kt // 4
                qstart = 512 * h0
                ew = HWP - qstart
                et = p1.tile([128, ew], BF16, name=f"Eatt{kt}", tag=f"Eatt{kt}")
                E_att.append((et, qstart))
                zpad = (kt % 4) * 128
                spans = [(128 * kt, 512 * (h0 + 1))]
                if h0 == 0:
                    spans.append((512, 1024))
                for ga, gb_ in spans:
                    ps = pm_tile()
                    w_ = gb_ - ga
                    nc.tensor.matmul(
                        ps[:, 0:w_],
                        lhsT=v["K_sb"][0:KD, kt * 128 : (kt + 1) * 128],
                        rhs=v["Q_sb"][0:KD, ga:gb_],
                        start=True,
                        stop=True,
                    )
                    nc.scalar.activation(
                        et[:, ga - qstart : gb_ - qstart], ps[:, 0:w_], AF.Exp
                    )
                nc.vector.tensor_tensor(
                    et[:, zpad : zpad + 128],
                    et[:, zpad : zpad + 128],
                    w1("masks")[:, 0:128],
                    op=OP.mult,
                )
            v["E_att"] = E_att
            # V^T (+ ones row for softmax row sums) via PE transpose
            VT = []
            for pc in range(8):
                pst = pT.tile([128, 512], BF16, name="St", tag="St")
                nc.tensor.transpose(
                    pst[:, 0:80],
                    v["V_sb"][:VD, pc * 128 : (pc + 1) * 128],
                    w1("ident80")[:80, :80],
                )
                vt = p1.tile([128, 97], BF16, name=f"VT{pc}", tag=f"VT{pc}")
                nc.vector.tensor_copy(vt[:, 0:80], pst[:, 0:80])
                if s < 1:  # cols 80:97 (zeros + ones row) never change
                    nc.gpsimd.memset(vt[:, 80:96], 0.0)
                    nc.gpsimd.memset(vt[:, 96:97], 1.0)
                VT.append(vt)
            v["VT"] = VT

            # AV accumulate per 512-half; row 96 = sum_k E (softmax denom).
            # Separate banks per half; rhs starts at the k-tile's first
            # nonzero column (cols left of it are never written or read).
            att = p1.tile([VD, HWP], BF16, name="att", tag="att")
            for qc in range(2):
                qsl = slice(qc * 512, (qc + 1) * 512)
                pav = pAV.tile([97, 512], F32, name=f"AV{qc}", tag=f"AV{qc}")
                kts = [kt for kt in range(8) if 128 * kt < (qc + 1) * 512]
                for i, kt in enumerate(kts):
                    et, qstart = v["E_att"][kt]
                    zpad = (kt % 4) * 128
                    c0 = qc * 512 - qstart
                    lo = max(c0, zpad)
                    nc.tensor.matmul(
                        pav[:97, lo - c0 : 512],
                        lhsT=v["VT"][kt][:, 0:97],
                        rhs=et[:, lo : c0 + 512],
                        start=(i == 0),
                        stop=(i == len(kts) - 1),
                        skip_group_check=(lo != c0),
                    )
                # att = AV[0:80] / ((1+eps)*R + eps*(1024-q))
                den_t = p1.tile([1, 512], F32, name=f"den{qc}", tag=f"den{qc}")
                nc.vector.scalar_tensor_tensor(
                    den_t[0:1, :], pav[96:97, 0:512], 1.0 + EPS,
                    w1("eps_nzq")[0:1, qsl], OP.mult, OP.add,
                )
                nc.vector.reciprocal_approx_fast(den_t[0:1, :], den_t[0:1, :])
                attb = p1.tile([VD, 512], F32, name=f"attb{qc}", tag=f"attb{qc}")
                nc.gpsimd.partition_broadcast(attb[:VD, :], den_t[0:1, :])
                nc.vector.tensor_tensor(
                    att[:VD, qsl], pav[0:VD, 0:512], attb[:VD, :], op=OP.mult
                )

            # att stream Sa: pos/neg tiles [80]; single-exp elu, att -> 1-att
            # (bufs=2: Sa crosses the stage boundary into st_out1 under the
            # two-sample interleave)
            Sa_p = p1.tile([VD, HWP], BF16, name="Sap", tag="Sap", bufs=2)
            Sa_n = p1.tile([VD, HWP], BF16, name="San", tag="San", bufs=2)
            aha = p1.tile([VD, HWP], BF16, name="aha", tag="aha")
            eea = p1.tile([VD, HWP], BF16, name="eea", tag="eea")
            nc.vector.tensor_scalar(Sa_p[:VD, :], att[:VD, :], 1.0, None, OP.add)
            nc.vector.tensor_scalar(
                att[:VD, :], Sa_p[:VD, :], -1.0, 2.0, OP.mult, OP.add
            )
            nc.vector.tensor_tensor(
                aha[:VD, :], Sa_p[:VD, :], att[:VD, :], op=OP.max
            )
            nc.scalar.activation(
                eea[:VD, :], aha[:VD, :], AF.Exp, scale=-1.0, bias=1.0
            )
            nc.vector.tensor_tensor(
                Sa_p[:VD, :], Sa_p[:VD, :], eea[:VD, :], op=OP.max
            )
            nc.vector.tensor_tensor(Sa_n[:VD, :], att[:VD, :], eea[:VD, :], op=OP.max)
            v["Sa_p"], v["Sa_n"] = Sa_p, Sa_n

        def st_out1(s, v):
            # bufs=2: ho crosses into st_out2 under the two-sample interleave
            ho = alloc_row_tiles(p1, NF, HWP, "ho", bufs=2)
            for nco in range(0, HWP, 512):
                nsl = slice(nco, nco + 512)
                psums = []
                for m_off, m_len in chunks(NF):
                    psums.append((pm_tile(), m_off, m_len))
                emit_mm(
                    psums,
                    [
                        (W["wo1a_t"], v["Sc"]),
                        (W["wo1bp"], [(v["Sa_p"], VD)]),
                        (W["wo1bn"], [(v["Sa_n"], VD)]),
                    ],
                    nsl,
                )
                copy_h(psums, 0, NF, ho, nsl, w1("bo1"))
            v["ho"] = ho

        def st_out2(s, v):
            So = alloc_row_tiles(p1, PAD + NF, HWP, "So")
            if s < 1:
                nc.gpsimd.memset(So[1][0][32:64, :], 0.0)
            emit_elu(v["ho"], NF, So)
            Opre = alloc_row_tiles(p1, NF, HWP, "Opre")
            O0 = p1.tile([128, HWP], BF16, name="O0", tag="O0")
            O1 = p1.tile([32, HWP], BF16, name="O1", tag="O1")
            for nco in range(0, HWP, 512):
                nsl = slice(nco, nco + 512)
                psums = []
                for m_off, m_len in chunks(PAD + NF):
                    psums.append((pm_tile(), m_off, m_len))
                emit_mm(psums, [(W["woo_t"], So)], nsl)
                emit_gate(psums, NF, w1("boo"), w1("bgo"), Opre, nsl)
            C0, C1 = v["C_tiles"][0][0], v["C_tiles"][1][0]
            nc.vector.tensor_tensor(O0[:, :], Opre[0][0][:, :], C0[:, :], op=OP.add)
            nc.vector.tensor_tensor(
                O1[:32, :], Opre[1][0][:32, :], C1[0:32, :], op=OP.add
            )
            nc.gpsimd.dma_start(out_d[s, 0:128, :], O0[:, :])
            nc.gpsimd.dma_start(out_d[s, 128:160, :], O1[:32, :])

        stages = [
            st_input,
            st_nin1,
            make_grn("k", "hk", CK, "wok_t", "bok", "bgk"),
            make_grn("q", "hq", CQ, "woq_t", "boq", "bgq"),
            make_grn("v", "hv", CK, "wov_t", "bov", "bgv"),
            st_proj,
            st_attn,
            st_out1,
            st_out2,
        ]
        assert ns % 2 == 0
        for pair in range(ns // 2):
            va, vb = {}, {}
            for stage in stages:
                stage(2 * pair, va)
                stage(2 * pair + 1, vb)

    nc.compile()
    return nc


_NC_CACHE = {}


def _get_nc():
    if "nc" not in _NC_CACHE:
        _NC_CACHE["nc"] = build_nc()
    return _NC_CACHE["nc"]


def make_in_maps(inputs):
    inp = {
        k: np.ascontiguousarray(np.asarray(v), dtype=np.float32)
        for k, v in inputs.items()
    }
    p = prep_weights(inp)
    for k, sshape in WSHAPES.items():
        assert p[k].shape == sshape, (k, p[k].shape, sshape)

    x = inp["x"].reshape(N, XD, HWP).astype(BF)
    ul = inp["ul"].reshape(N, NF, HWP).astype(BF)
    b = inp["b"].reshape(N, 2 * XD, HWP).astype(BF)
    wpb, wpf = pack_weights(p)

    in_maps = []
    for c in range(NCORES):
        sl = slice(c * NS, (c + 1) * NS)
        m = {
            "x": x[sl], "ul": ul[sl], "b": b[sl],
            "wpb": wpb, "wpf": wpf, "eps_nzq": p["eps_nzq"],
        }
        in_maps.append(m)
    return in_maps


def kernel(**inputs):
    in_maps = make_in_maps(inputs)
    nc = _get_nc()
    res = run_bass_kernel_spmd(nc, in_maps, core_ids=list(range(NCORES)))
    out = np.concatenate([r["out"] for r in res.results], axis=0)
    return out.reshape(N, NF, 32, 32)


if __name__ == "__main__":
    import reference as R

    inputs = {k: np.asarray(v) for k, v in R.setup_inputs().items()}
    got = kernel(**inputs)
    exp = np.asarray(R.reference(**R.setup_inputs()))
    err = np.abs(got - exp)
    print("max abs err:", err.max(), "rel:", err.max() / np.abs(exp).max())

